# revision 5
# baseline (speedup 1.0000x reference)
"""Trainium2 Bass kernel for nn_DecoderLayer_19851338842283.

Strategy (8 NeuronCores): data-parallel over batch (4) x tensor-parallel (2)
over heads (8 each) + mlp_dim (2048 each).  Each core computes partial
attention + MLP outputs for one batch; the host sums the two tensor-parallel
partials and adds the residual.  No on-device collectives.

Device-side layout is fully transpose-free:
  - host passes x^T [E, L], so projections produce q^T/k^T in [head*d, L]
    layout and v in natural [L, head*d] layout directly.
  - scores are computed transposed ([k, q]); softmax denominators come from a
    fused ones-column in the v operand (M=65 matmuls); the T5 relative-
    position bias + causal mask is a host-precomputed Toeplitz band [128,640]
    per head (exact for |q-k| <= 511; bias is constant for q-k >= 113, which
    cancels in softmax) added on VectorE before the exponent.
  - all matmuls run as float32r (TF32-like, full PE rate at N>=512).
"""

import numpy as np

import concourse.bacc as bacc
import concourse.mybir as mybir
import concourse.tile as tile
from concourse.bass_utils import run_bass_kernel_spmd

F32 = mybir.dt.float32
F32R = mybir.dt.float32r
Act = mybir.ActivationFunctionType
Alu = mybir.AluOpType

B, L, E, H, D, F = 4, 2048, 1024, 16, 64, 4096
HC = H // 2          # heads per core = 8
FC = F // 2          # mlp dim per core = 2048
NCORES = 8
ET = E // 128        # 8  e-tiles
LT = L // 128        # 16 l-tiles
PT = HC * D // 128   # 4  head-pair tiles
FT = FC // 128       # 16 f-tiles
NSUP = L // 512      # 4  q-supers
BAND_OFF = 384
BAND_W = 640
NUM_BUCKETS = 32


def _build(causal: bool):
    nc = bacc.Bacc("TRN2", target_bir_lowering=False, debug=False,
                   num_devices=NCORES)
    xT_d = nc.dram_tensor("xT", [E, L], F32R, kind="ExternalInput").ap()
    wq_d = nc.dram_tensor("wq", [E, HC * D], F32R, kind="ExternalInput").ap()
    wk_d = nc.dram_tensor("wk", [E, HC * D], F32R, kind="ExternalInput").ap()
    wv_d = nc.dram_tensor("wv", [E, HC * D], F32R, kind="ExternalInput").ap()
    wo_d = nc.dram_tensor("wo", [HC * D, E], F32R, kind="ExternalInput").ap()
    wi_d = nc.dram_tensor("wi", [E, FC], F32R, kind="ExternalInput").ap()
    wmo_d = nc.dram_tensor("wmo", [FC, E], F32R, kind="ExternalInput").ap()
    band_d = nc.dram_tensor("band", [128, HC, BAND_W], F32,
                            kind="ExternalInput").ap()
    bfut_d = nc.dram_tensor("bfut", [128, HC], F32, kind="ExternalInput").ap()
    attn_d = nc.dram_tensor("attn_out", [L, E], F32, kind="ExternalOutput").ap()
    mlp_d = nc.dram_tensor("mlp_out", [L, E], F32, kind="ExternalOutput").ap()
    hT_d = nc.dram_tensor("hT_scr", [FT, 128, L], F32R).ap()

    with tile.TileContext(nc) as tc:
        with (
            tc.tile_pool(name="pps", bufs=4, space="PSUM") as pps,
            tc.tile_pool(name="pctx", bufs=4, space="PSUM") as pctx,
            tc.tile_pool(name="pob", bufs=4) as pob,
        ):
            with (
                tc.tile_pool(name="pqk", bufs=2 * PT) as pqk,
                tc.tile_pool(name="pva", bufs=1) as pva,
            ):
                qT = [pqk.tile([128, L], F32R, tag="qk", name=f"qT{i}") for i in range(PT)]
                kT = [pqk.tile([128, L], F32R, tag="qk", name=f"kT{i}") for i in range(PT)]
                va_all = pva.tile([128, LT * HC * 65], F32R, tag="va",
                                  name="va_all")

                # ---------------- phase 1: projections + MLP-in ----------
                with tc.tile_pool(name="pxT", bufs=ET) as pxT:
                    xt = [pxT.tile([128, L], F32R, tag="xT", name=f"xt{i}")
                          for i in range(ET)]
                    for t in range(ET):
                        nc.sync.dma_start(xt[t][:], xT_d[128 * t:128 * t + 128, :])

                    # q and k projections (q scaled by 1/sqrt(D)=1/8)
                    with tc.tile_pool(name="pw", bufs=2) as pw:
                        for w_d, dst, scale in ((wq_d, qT, 0.125),
                                                (wk_d, kT, None)):
                            w3 = w_d.rearrange("(et ep) hd -> ep et hd", ep=128)
                            for p in range(PT):
                                ws = pw.tile([128, ET, 128], F32R, tag="w")
                                nc.sync.dma_start(
                                    ws[:], w3[:, :, 128 * p:128 * p + 128])
                                for c in range(NSUP):
                                    acc = pps.tile([128, 512], F32, tag="ps")
                                    for t in range(ET):
                                        nc.tensor.matmul(
                                            acc[:], ws[:, t, :],
                                            xt[t][:, 512 * c:512 * c + 512],
                                            start=(t == 0), stop=(t == ET - 1))
                                    o = dst[p][:, 512 * c:512 * c + 512]
                                    if scale is None:
                                        nc.scalar.copy(o, acc[:])
                                    else:
                                        nc.scalar.mul(o, acc[:], scale)

                    # v projection into [L, hd] with interleaved ones columns
                    with tc.tile_pool(name="pwv", bufs=1) as pwv:
                        wvs = pwv.tile([128, ET, 512], F32R, tag="wv")
                        nc.sync.dma_start(
                            wvs[:],
                            wv_d.rearrange("(et ep) hd -> ep et hd", ep=128))
                        ones_c = nc.const_aps.tensor(1.0, [128, HC, 1], F32)
                        for lt in range(LT):
                            acc = pps.tile([128, 512], F32, tag="ps")
                            for t in range(ET):
                                nc.tensor.matmul(
                                    acc[:], xt[t][:, 128 * lt:128 * lt + 128],
                                    wvs[:, t, :],
                                    start=(t == 0), stop=(t == ET - 1))
                            va3 = va_all[:, 520 * lt:520 * lt + 520].rearrange(
                                "p (h c) -> p h c", h=HC)
                            nc.vector.tensor_copy(
                                va3[:, :, 0:64],
                                acc[:].rearrange("p (h c) -> p h c", h=HC))
                            nc.vector.tensor_copy(va3[:, :, 64:65], ones_c)

                    # MLP up-projection + relu, staged to HBM scratch
                    with (
                        tc.tile_pool(name="pwi", bufs=2) as pwi,
                        tc.tile_pool(name="phT", bufs=4) as phT,
                    ):
                        wi3 = wi_d.rearrange("(et ep) f -> ep et f", ep=128)
                        for ft in range(FT):
                            wis = pwi.tile([128, ET, 128], F32R, tag="wi")
                            nc.sync.dma_start(
                                wis[:], wi3[:, :, 128 * ft:128 * ft + 128])
                            for c in range(NSUP):
                                acc = pps.tile([128, 512], F32, tag="ps")
                                for t in range(ET):
                                    nc.tensor.matmul(
                                        acc[:], wis[:, t, :],
                                        xt[t][:, 512 * c:512 * c + 512],
                                        start=(t == 0), stop=(t == ET - 1))
                                hst = phT.tile([128, 512], F32R, tag="hT")
                                nc.scalar.activation(hst[:], acc[:], Act.Relu)
                                nc.sync.dma_start(
                                    hT_d[ft, :, 512 * c:512 * c + 512], hst[:])

                # ---------------- phase 2: attention ---------------------
                with (
                    tc.tile_pool(name="pband", bufs=1) as pband,
                    tc.tile_pool(name="pwo", bufs=1) as pwo,
                    tc.tile_pool(name="pexp", bufs=4) as pexp,
                    tc.tile_pool(name="pct", bufs=6) as pct,
                    tc.tile_pool(name="prr", bufs=2) as prr,
                    tc.tile_pool(name="prs", bufs=2) as prs,
                    tc.tile_pool(name="prb", bufs=2) as prb,
                    tc.tile_pool(name="pth", bufs=2) as pth,
                ):
                    band_sb = pband.tile([128, HC * BAND_W], F32, tag="band")
                    nc.sync.dma_start(
                        band_sb[:],
                        band_d.rearrange("p h w -> p (h w)"))
                    band3 = band_sb[:].rearrange("p (h w) -> p h w", h=HC)
                    bfut_sb = pband.tile([128, HC], F32, tag="bfut")
                    nc.sync.dma_start(bfut_sb[:], bfut_d)
                    wos = pwo.tile([128, PT, E], F32R, tag="wo")
                    nc.sync.dma_start(
                        wos[:], wo_d.rearrange("(pt pp) e -> pp pt e", pp=128))

                    for s in range(NSUP):
                        qs = 512 * s
                        ktiles = 4 * (s + 1) if causal else LT
                        cts = []
                        for p in range(PT):
                            ct = pct.tile([128, 512], F32R, tag="ct")
                            cts.append(ct)
                            cpa = pctx.tile([65, 512], F32, tag="ctx")
                            cpb = pctx.tile([65, 512], F32, tag="ctx")
                            for kt in range(ktiles):
                                k0 = 128 * kt
                                sp = []
                                for half, tp in ((0, (0, 0)), (1, (64, 0))):
                                    ps_t = pps.tile([128, 512], F32, tag="ps")
                                    sp.append(ps_t)
                                    r0 = 64 * half
                                    nc.tensor.matmul(
                                        ps_t[:],
                                        kT[p][r0:r0 + 64, k0:k0 + 128],
                                        qT[p][r0:r0 + 64, qs:qs + 512],
                                        start=True, stop=True,
                                        tile_position=tp)
                                o_lo = max(k0 - BAND_OFF, qs)
                                o_hi = min(k0 + 256, qs + 512)
                                ul = (min(max(k0 - BAND_OFF - qs, 0), 512)
                                      if not causal else 0)
                                for half in (0, 1):
                                    h = 2 * p + half
                                    ps_t = sp[half]
                                    full_fut = (not causal) and o_hi <= o_lo \
                                        and k0 > qs + 511
                                    if ul > 0 and not full_fut:
                                        nc.vector.tensor_scalar_add(
                                            ps_t[:, 0:ul], ps_t[:, 0:ul],
                                            bfut_sb[:, h:h + 1])
                                    if o_hi > o_lo:
                                        psl = slice(o_lo - qs, o_hi - qs)
                                        bsl = slice(o_lo - (k0 - BAND_OFF),
                                                    o_hi - (k0 - BAND_OFF))
                                        nc.vector.tensor_tensor(
                                            ps_t[:, psl], ps_t[:, psl],
                                            band3[:, h, bsl], Alu.add)
                                    ea = pexp.tile([128, 512], F32R, tag="exp")
                                    if full_fut:
                                        nc.scalar.activation(
                                            ea[:], ps_t[:], Act.Exp,
                                            bias=bfut_sb[:, h:h + 1])
                                    else:
                                        nc.scalar.activation(
                                            ea[:], ps_t[:], Act.Exp)
                                    cp = cpa if half == 0 else cpb
                                    base = 520 * kt + 65 * h
                                    nc.tensor.matmul(
                                        cp[0:65, :],
                                        va_all[:, base:base + 65],
                                        ea[:],
                                        start=(kt == 0),
                                        stop=(kt == ktiles - 1))
                            # normalize by softmax denominator and pack ct
                            for half, cp in ((0, cpa), (1, cpb)):
                                rr = prr.tile([128, 512], F32, tag="rr")
                                nc.vector.reciprocal(rr[64:65, :],
                                                     cp[64:65, :])
                                rs = prs.tile([1, 512], F32, tag="rs")
                                nc.sync.dma_start(rs[0:1, :], rr[64:65, :])
                                rb = prb.tile([64, 512], F32, tag="rb")
                                nc.gpsimd.partition_broadcast(
                                    rb[:], rs[0:1, :])
                                if half == 0:
                                    nc.vector.tensor_tensor(
                                        ct[0:64, :], cp[0:64, :], rb[:],
                                        Alu.mult)
                                else:
                                    th = pth.tile([64, 512], F32R, tag="th")
                                    nc.vector.tensor_tensor(
                                        th[:], cp[0:64, :], rb[:], Alu.mult)
                                    nc.sync.dma_start(ct[64:128, :], th[:])
                        # output projection for this q-super
                        for qt in range(4):
                            for ec in range(2):
                                acc = pps.tile([128, 512], F32, tag="ps")
                                for p in range(PT):
                                    nc.tensor.matmul(
                                        acc[:],
                                        cts[p][:, 128 * qt:128 * qt + 128],
                                        wos[:, p, 512 * ec:512 * ec + 512],
                                        start=(p == 0), stop=(p == PT - 1))
                                ob = pob.tile([128, 512], F32, tag="ob")
                                nc.vector.tensor_copy(ob[:], acc[:])
                                nc.sync.dma_start(
                                    attn_d[qs + 128 * qt:qs + 128 * qt + 128,
                                           512 * ec:512 * ec + 512], ob[:])

            # ---------------- phase 3: MLP down-projection ---------------
            with (
                tc.tile_pool(name="pwmo", bufs=FT) as pwmo,
                tc.tile_pool(name="phin", bufs=3) as phin,
            ):
                wmo3 = wmo_d.rearrange("(ft fp) e -> fp ft e", fp=128)
                wms = []
                for ft in range(FT):
                    wm = pwmo.tile([128, E], F32R, tag="wmo")
                    nc.sync.dma_start(wm[:], wmo3[:, ft, :])
                    wms.append(wm)
                hT3 = hT_d.rearrange("ft fp l -> fp ft l")
                for lt in range(LT):
                    hins = phin.tile([128, FT, 128], F32R, tag="hin")
                    nc.sync.dma_start(
                        hins[:], hT3[:, :, 128 * lt:128 * lt + 128])
                    for ec in range(2):
                        acc = pps.tile([128, 512], F32, tag="ps")
                        for ft in range(FT):
                            nc.tensor.matmul(
                                acc[:], hins[:, ft, :],
                                wms[ft][:, 512 * ec:512 * ec + 512],
                                start=(ft == 0), stop=(ft == FT - 1))
                        ob = pob.tile([128, 512], F32, tag="ob")
                        nc.vector.tensor_copy(ob[:], acc[:])
                        nc.sync.dma_start(
                            mlp_d[128 * lt:128 * lt + 128,
                                  512 * ec:512 * ec + 512], ob[:])

    nc.compile()
    return nc


_NC_CACHE = {}


def _get_nc(causal: bool):
    if causal not in _NC_CACHE:
        _NC_CACHE[causal] = _build(causal)
    return _NC_CACHE[causal]


def _bucket(n):
    """T5 relative-position bucket (causal), exact numpy replica of the
    jax fp32 reference computation."""
    n = np.asarray(n)
    nf = np.maximum(n.astype(np.float32), np.float32(1.0))
    v = np.log(nf / np.float32(16.0)).astype(np.float32)
    v = (v / np.float32(np.log(8.0))) * np.float32(16.0)
    val_large = 16 + v.astype(np.int32)
    val_large = np.minimum(val_large, NUM_BUCKETS - 1)
    return np.where(n < 16, n, val_large)


def _make_band(rel_emb, heads, causal):
    """band[i, hl, j] = adjustment for distance d = j - BAND_OFF - i.

    d < 0   : -30000 (causal mask) or rel_emb[0]-rel_emb[31] (dense)
    0..112  : rel_emb[bucket(d)] - rel_emb[31]
    >= 113  : 0   (bucket 31 everywhere; constant per row cancels in softmax)
    """
    d = np.arange(-(BAND_OFF + 127), 256)          # all possible j - OFF - i
    pos = np.maximum(d, 0)
    bv = rel_emb[_bucket(pos)][:, heads] - rel_emb[NUM_BUCKETS - 1][heads]
    bv = np.where(d[:, None] >= 113, np.float32(0.0), bv)
    if causal:
        bv = np.where(d[:, None] < 0, np.float32(-30000.0), bv)
    else:
        fut = rel_emb[0][heads] - rel_emb[NUM_BUCKETS - 1][heads]
        bv = np.where(d[:, None] < 0, fut[None, :], bv)
    i = np.arange(128)[:, None]
    j = np.arange(BAND_W)[None, :]
    idx = (j - BAND_OFF - i) + (BAND_OFF + 127)
    return bv.astype(np.float32)[idx]          # [128, BAND_W, HC]


def run(inputs, wq, wk, wv, wo, wi, wmo, rel_emb, decoder_mask, trace=False):
    inputs = np.asarray(inputs, dtype=np.float32)
    wq = np.asarray(wq, dtype=np.float32)
    wk = np.asarray(wk, dtype=np.float32)
    wv = np.asarray(wv, dtype=np.float32)
    wo = np.asarray(wo, dtype=np.float32)
    wi = np.asarray(wi, dtype=np.float32)
    wmo = np.asarray(wmo, dtype=np.float32)
    rel_emb = np.asarray(rel_emb, dtype=np.float32)
    mask = np.asarray(decoder_mask).reshape(L, L)

    tril = np.tril(np.ones((L, L), dtype=bool))
    if np.array_equal(mask, tril):
        causal = True
    elif mask.all():
        causal = False
    else:
        raise NotImplementedError("only causal or all-true masks supported")

    nc = _get_nc(causal)

    in_maps = []
    for c in range(NCORES):
        b, g = divmod(c, 2)
        heads = np.arange(HC * g, HC * (g + 1))
        band = _make_band(rel_emb, heads, causal)       # [128, BAND_W, HC]
        band = np.ascontiguousarray(band.transpose(0, 2, 1))  # [128, HC, W]
        bfut = np.broadcast_to(
            (rel_emb[0][heads] - rel_emb[NUM_BUCKETS - 1][heads])
            .astype(np.float32), (128, HC)).copy()
        in_maps.append(dict(
            xT=np.ascontiguousarray(inputs[b].T),
            wq=np.ascontiguousarray(wq[:, heads, :]).reshape(E, HC * D),
            wk=np.ascontiguousarray(wk[:, heads, :]).reshape(E, HC * D),
            wv=np.ascontiguousarray(wv[:, heads, :]).reshape(E, HC * D),
            wo=np.ascontiguousarray(wo[heads]).reshape(HC * D, E),
            wi=np.ascontiguousarray(wi[:, FC * g:FC * (g + 1)]),
            wmo=np.ascontiguousarray(wmo[FC * g:FC * (g + 1), :]),
            band=band,
            bfut=bfut,
        ))

    res = run_bass_kernel_spmd(nc, in_maps, list(range(NCORES)), trace=trace)
    out = np.empty((B, L, E), dtype=np.float32)
    for b in range(B):
        out[b] = (inputs[b]
                  + res.results[2 * b]["attn_out"]
                  + res.results[2 * b]["mlp_out"]
                  + res.results[2 * b + 1]["attn_out"]
                  + res.results[2 * b + 1]["mlp_out"])
    return out, res


def kernel(**inputs):
    out, _ = run(**inputs)
    return out


# revision 6
# speedup vs baseline: 136.9455x; 136.9455x over previous
"""Trainium2 Bass kernel for nn_DecoderLayer_19851338842283.

Strategy (8 NeuronCores): data-parallel over batch (4) x tensor-parallel (2)
over heads (8 each) + mlp_dim (2048 each).  Each core computes partial
attention + MLP outputs for one batch; the host sums the two tensor-parallel
partials and adds the residual.  No on-device collectives.

Device-side layout is fully transpose-free:
  - host passes x^T [E, L], so projections produce q^T/k^T in [head*d, L]
    layout and v in natural [L, head*d] layout directly.
  - scores are computed transposed ([k, q]); softmax denominators come from a
    fused ones-column in the v operand (M=65 matmuls); the T5 relative-
    position bias + causal mask is a host-precomputed Toeplitz band [128,640]
    per head (exact for |q-k| <= 511; bias is constant for q-k >= 113, which
    cancels in softmax) added on VectorE before the exponent.
  - all matmuls run as float32r (TF32-like, full PE rate at N>=512).
"""

import numpy as np

import concourse.bacc as bacc
import concourse.mybir as mybir
import concourse.tile as tile
from concourse.bass_utils import run_bass_kernel_spmd

F32 = mybir.dt.float32
F32R = mybir.dt.float32r
Act = mybir.ActivationFunctionType
Alu = mybir.AluOpType

B, L, E, H, D, F = 4, 2048, 1024, 16, 64, 4096
HC = H // 2          # heads per core = 8
FC = F // 2          # mlp dim per core = 2048
NCORES = 8
ET = E // 128        # 8  e-tiles
LT = L // 128        # 16 l-tiles
PT = HC * D // 128   # 4  head-pair tiles
FT = FC // 128       # 16 f-tiles
NSUP = L // 512      # 4  q-supers
BAND_OFF = 384
BAND_W = 640
NUM_BUCKETS = 32


def _build(causal: bool):
    nc = bacc.Bacc("TRN2", target_bir_lowering=False, debug=False,
                   num_devices=NCORES)
    xT_d = nc.dram_tensor("xT", [E, L], F32R, kind="ExternalInput").ap()
    wq_d = nc.dram_tensor("wq", [E, HC * D], F32R, kind="ExternalInput").ap()
    wk_d = nc.dram_tensor("wk", [E, HC * D], F32R, kind="ExternalInput").ap()
    wv_d = nc.dram_tensor("wv", [E, HC * D], F32R, kind="ExternalInput").ap()
    wo_d = nc.dram_tensor("wo", [HC * D, E], F32R, kind="ExternalInput").ap()
    wi_d = nc.dram_tensor("wi", [E, FC], F32R, kind="ExternalInput").ap()
    wmo_d = nc.dram_tensor("wmo", [FC, E], F32R, kind="ExternalInput").ap()
    band_d = nc.dram_tensor("band", [128, HC, BAND_W], F32,
                            kind="ExternalInput").ap()
    bfut_d = nc.dram_tensor("bfut", [128, HC], F32, kind="ExternalInput").ap()
    attn_d = nc.dram_tensor("attn_out", [L, E], F32, kind="ExternalOutput").ap()
    mlp_d = nc.dram_tensor("mlp_out", [L, E], F32, kind="ExternalOutput").ap()
    hT_d = nc.dram_tensor("hT_scr", [FT, 128, L], F32R).ap()

    with tile.TileContext(nc) as tc:
        with (
            tc.tile_pool(name="pps", bufs=4, space="PSUM") as pps,
            tc.tile_pool(name="pctx", bufs=4, space="PSUM") as pctx,
            tc.tile_pool(name="pob", bufs=4) as pob,
        ):
            with (
                tc.tile_pool(name="pqk", bufs=2 * PT) as pqk,
                tc.tile_pool(name="pva", bufs=1) as pva,
            ):
                qT = [pqk.tile([128, L], F32R, tag="qk", name=f"qT{i}") for i in range(PT)]
                kT = [pqk.tile([128, L], F32R, tag="qk", name=f"kT{i}") for i in range(PT)]
                va_all = pva.tile([128, LT * HC * 65], F32R, tag="va",
                                  name="va_all")

                # ---------------- phase 1: projections + MLP-in ----------
                with tc.tile_pool(name="pxT", bufs=ET) as pxT:
                    xt = [pxT.tile([128, L], F32R, tag="xT", name=f"xt{i}")
                          for i in range(ET)]
                    for t in range(ET):
                        nc.sync.dma_start(xt[t][:], xT_d[128 * t:128 * t + 128, :])

                    # q and k projections (q scaled by 1/sqrt(D)=1/8)
                    with tc.tile_pool(name="pw", bufs=2) as pw:
                        for w_d, dst, scale in ((wq_d, qT, 0.125),
                                                (wk_d, kT, None)):
                            w3 = w_d.rearrange("(et ep) hd -> ep et hd", ep=128)
                            for p in range(PT):
                                ws = pw.tile([128, ET, 128], F32R, tag="w")
                                nc.sync.dma_start(
                                    ws[:], w3[:, :, 128 * p:128 * p + 128])
                                for c in range(NSUP):
                                    acc = pps.tile([128, 512], F32, tag="ps")
                                    for t in range(ET):
                                        nc.tensor.matmul(
                                            acc[:], ws[:, t, :],
                                            xt[t][:, 512 * c:512 * c + 512],
                                            start=(t == 0), stop=(t == ET - 1))
                                    o = dst[p][:, 512 * c:512 * c + 512]
                                    if scale is None:
                                        nc.scalar.copy(o, acc[:])
                                    else:
                                        nc.scalar.mul(o, acc[:], scale)

                    # v projection into [L, hd] with interleaved ones columns
                    with tc.tile_pool(name="pwv", bufs=1) as pwv:
                        wvs = pwv.tile([128, ET, 512], F32R, tag="wv")
                        nc.sync.dma_start(
                            wvs[:],
                            wv_d.rearrange("(et ep) hd -> ep et hd", ep=128))
                        ones_c = nc.const_aps.tensor(1.0, [128, HC, 1], F32)
                        for lt in range(LT):
                            acc = pps.tile([128, 512], F32, tag="ps")
                            for t in range(ET):
                                nc.tensor.matmul(
                                    acc[:], xt[t][:, 128 * lt:128 * lt + 128],
                                    wvs[:, t, :],
                                    start=(t == 0), stop=(t == ET - 1))
                            va3 = va_all[:, 520 * lt:520 * lt + 520].rearrange(
                                "p (h c) -> p h c", h=HC)
                            nc.vector.tensor_copy(
                                va3[:, :, 0:64],
                                acc[:].rearrange("p (h c) -> p h c", h=HC))
                            nc.vector.tensor_copy(va3[:, :, 64:65], ones_c)

                    # MLP up-projection + relu, staged to HBM scratch
                    with (
                        tc.tile_pool(name="pwi", bufs=2) as pwi,
                        tc.tile_pool(name="phT", bufs=4) as phT,
                    ):
                        wi3 = wi_d.rearrange("(et ep) f -> ep et f", ep=128)
                        for ft in range(FT):
                            wis = pwi.tile([128, ET, 128], F32R, tag="wi")
                            nc.sync.dma_start(
                                wis[:], wi3[:, :, 128 * ft:128 * ft + 128])
                            for c in range(NSUP):
                                acc = pps.tile([128, 512], F32, tag="ps")
                                for t in range(ET):
                                    nc.tensor.matmul(
                                        acc[:], wis[:, t, :],
                                        xt[t][:, 512 * c:512 * c + 512],
                                        start=(t == 0), stop=(t == ET - 1))
                                hst = phT.tile([128, 512], F32R, tag="hT")
                                nc.scalar.activation(hst[:], acc[:], Act.Relu)
                                nc.sync.dma_start(
                                    hT_d[ft, :, 512 * c:512 * c + 512], hst[:])

                # ---------------- phase 2: attention ---------------------
                with (
                    tc.tile_pool(name="pband", bufs=1) as pband,
                    tc.tile_pool(name="pwo", bufs=1) as pwo,
                    tc.tile_pool(name="pexp", bufs=4) as pexp,
                    tc.tile_pool(name="pct", bufs=6) as pct,
                    tc.tile_pool(name="prr", bufs=2) as prr,
                    tc.tile_pool(name="prs", bufs=2) as prs,
                    tc.tile_pool(name="prb", bufs=2) as prb,
                    tc.tile_pool(name="pth", bufs=2) as pth,
                ):
                    band_sb = pband.tile([128, HC * BAND_W], F32, tag="band")
                    nc.sync.dma_start(
                        band_sb[:],
                        band_d.rearrange("p h w -> p (h w)"))
                    band3 = band_sb[:].rearrange("p (h w) -> p h w", h=HC)
                    bfut_sb = pband.tile([128, HC], F32, tag="bfut")
                    nc.sync.dma_start(bfut_sb[:], bfut_d)
                    wos = pwo.tile([128, PT, E], F32R, tag="wo")
                    nc.sync.dma_start(
                        wos[:], wo_d.rearrange("(pt pp) e -> pp pt e", pp=128))

                    for s in range(NSUP):
                        qs = 512 * s
                        ktiles = 4 * (s + 1) if causal else LT
                        cts = []
                        for p in range(PT):
                            ct = pct.tile([128, 512], F32R, tag="ct")
                            cts.append(ct)
                            cpa = pctx.tile([65, 512], F32, tag="ctx")
                            cpb = pctx.tile([65, 512], F32, tag="ctx")
                            for kt in range(ktiles):
                                k0 = 128 * kt
                                sp = []
                                for half, tp in ((0, (0, 0)), (1, (64, 0))):
                                    ps_t = pps.tile([128, 512], F32, tag="ps")
                                    sp.append(ps_t)
                                    r0 = 64 * half
                                    nc.tensor.matmul(
                                        ps_t[:],
                                        kT[p][r0:r0 + 64, k0:k0 + 128],
                                        qT[p][r0:r0 + 64, qs:qs + 512],
                                        start=True, stop=True,
                                        tile_position=tp)
                                o_lo = max(k0 - BAND_OFF, qs)
                                o_hi = min(k0 + 256, qs + 512)
                                ul = (min(max(k0 - BAND_OFF - qs, 0), 512)
                                      if not causal else 0)
                                for half in (0, 1):
                                    h = 2 * p + half
                                    ps_t = sp[half]
                                    full_fut = (not causal) and o_hi <= o_lo \
                                        and k0 > qs + 511
                                    if ul > 0 and not full_fut:
                                        nc.vector.tensor_scalar_add(
                                            ps_t[:, 0:ul], ps_t[:, 0:ul],
                                            bfut_sb[:, h:h + 1])
                                    if o_hi > o_lo:
                                        psl = slice(o_lo - qs, o_hi - qs)
                                        bsl = slice(o_lo - (k0 - BAND_OFF),
                                                    o_hi - (k0 - BAND_OFF))
                                        nc.vector.tensor_tensor(
                                            ps_t[:, psl], ps_t[:, psl],
                                            band3[:, h, bsl], Alu.add)
                                    ea = pexp.tile([128, 512], F32R, tag="exp")
                                    if full_fut:
                                        nc.scalar.activation(
                                            ea[:], ps_t[:], Act.Exp,
                                            bias=bfut_sb[:, h:h + 1])
                                    else:
                                        nc.scalar.activation(
                                            ea[:], ps_t[:], Act.Exp)
                                    cp = cpa if half == 0 else cpb
                                    base = 520 * kt + 65 * h
                                    nc.tensor.matmul(
                                        cp[0:65, :],
                                        va_all[:, base:base + 65],
                                        ea[:],
                                        start=(kt == 0),
                                        stop=(kt == ktiles - 1))
                            # normalize by softmax denominator and pack ct
                            for half, cp in ((0, cpa), (1, cpb)):
                                rr = prr.tile([128, 512], F32, tag="rr")
                                nc.vector.reciprocal(rr[64:65, :],
                                                     cp[64:65, :])
                                rs = prs.tile([1, 512], F32, tag="rs")
                                nc.sync.dma_start(rs[0:1, :], rr[64:65, :])
                                rb = prb.tile([64, 512], F32, tag="rb")
                                nc.gpsimd.partition_broadcast(
                                    rb[:], rs[0:1, :])
                                if half == 0:
                                    nc.vector.tensor_tensor(
                                        ct[0:64, :], cp[0:64, :], rb[:],
                                        Alu.mult)
                                else:
                                    th = pth.tile([64, 512], F32R, tag="th")
                                    nc.vector.tensor_tensor(
                                        th[:], cp[0:64, :], rb[:], Alu.mult)
                                    nc.sync.dma_start(ct[64:128, :], th[:])
                        # output projection for this q-super
                        for qt in range(4):
                            for ec in range(2):
                                acc = pps.tile([128, 512], F32, tag="ps")
                                for p in range(PT):
                                    nc.tensor.matmul(
                                        acc[:],
                                        cts[p][:, 128 * qt:128 * qt + 128],
                                        wos[:, p, 512 * ec:512 * ec + 512],
                                        start=(p == 0), stop=(p == PT - 1))
                                ob = pob.tile([128, 512], F32, tag="ob")
                                nc.vector.tensor_copy(ob[:], acc[:])
                                nc.sync.dma_start(
                                    attn_d[qs + 128 * qt:qs + 128 * qt + 128,
                                           512 * ec:512 * ec + 512], ob[:])

            # ---------------- phase 3: MLP down-projection ---------------
            with (
                tc.tile_pool(name="pwmo", bufs=FT) as pwmo,
                tc.tile_pool(name="phin", bufs=3) as phin,
            ):
                wmo3 = wmo_d.rearrange("(ft fp) e -> fp ft e", fp=128)
                wms = []
                for ft in range(FT):
                    wm = pwmo.tile([128, E], F32R, tag="wmo")
                    nc.sync.dma_start(wm[:], wmo3[:, ft, :])
                    wms.append(wm)
                hT3 = hT_d.rearrange("ft fp l -> fp ft l")
                for lt in range(LT):
                    hins = phin.tile([128, FT, 128], F32R, tag="hin")
                    nc.sync.dma_start(
                        hins[:], hT3[:, :, 128 * lt:128 * lt + 128])
                    for ec in range(2):
                        acc = pps.tile([128, 512], F32, tag="ps")
                        for ft in range(FT):
                            nc.tensor.matmul(
                                acc[:], hins[:, ft, :],
                                wms[ft][:, 512 * ec:512 * ec + 512],
                                start=(ft == 0), stop=(ft == FT - 1))
                        ob = pob.tile([128, 512], F32, tag="ob")
                        nc.vector.tensor_copy(ob[:], acc[:])
                        nc.sync.dma_start(
                            mlp_d[128 * lt:128 * lt + 128,
                                  512 * ec:512 * ec + 512], ob[:])

    nc.compile()
    return nc


_NC_CACHE = {}


def _get_nc(causal: bool):
    if causal not in _NC_CACHE:
        _NC_CACHE[causal] = _build(causal)
    return _NC_CACHE[causal]


def _bucket(n):
    """T5 relative-position bucket (causal), exact numpy replica of the
    jax fp32 reference computation."""
    n = np.asarray(n)
    nf = np.maximum(n.astype(np.float32), np.float32(1.0))
    v = np.log(nf / np.float32(16.0)).astype(np.float32)
    v = (v / np.float32(np.log(8.0))) * np.float32(16.0)
    val_large = 16 + v.astype(np.int32)
    val_large = np.minimum(val_large, NUM_BUCKETS - 1)
    return np.where(n < 16, n, val_large)


def _make_band(rel_emb, heads, causal):
    """band[i, hl, j] = adjustment for distance d = j - BAND_OFF - i.

    d < 0   : -30000 (causal mask) or rel_emb[0]-rel_emb[31] (dense)
    0..112  : rel_emb[bucket(d)] - rel_emb[31]
    >= 113  : 0   (bucket 31 everywhere; constant per row cancels in softmax)
    """
    d = np.arange(-(BAND_OFF + 127), 256)          # all possible j - OFF - i
    pos = np.maximum(d, 0)
    bv = rel_emb[_bucket(pos)][:, heads] - rel_emb[NUM_BUCKETS - 1][heads]
    bv = np.where(d[:, None] >= 113, np.float32(0.0), bv)
    if causal:
        bv = np.where(d[:, None] < 0, np.float32(-30000.0), bv)
    else:
        fut = rel_emb[0][heads] - rel_emb[NUM_BUCKETS - 1][heads]
        bv = np.where(d[:, None] < 0, fut[None, :], bv)
    i = np.arange(128)[:, None]
    j = np.arange(BAND_W)[None, :]
    idx = (j - BAND_OFF - i) + (BAND_OFF + 127)
    return bv.astype(np.float32)[idx]          # [128, BAND_W, HC]


def _prep_in_maps(inputs, wq, wk, wv, wo, wi, wmo, rel_emb, decoder_mask):
    inputs = np.asarray(inputs, dtype=np.float32)
    wq = np.asarray(wq, dtype=np.float32)
    wk = np.asarray(wk, dtype=np.float32)
    wv = np.asarray(wv, dtype=np.float32)
    wo = np.asarray(wo, dtype=np.float32)
    wi = np.asarray(wi, dtype=np.float32)
    wmo = np.asarray(wmo, dtype=np.float32)
    rel_emb = np.asarray(rel_emb, dtype=np.float32)
    mask = np.asarray(decoder_mask).reshape(L, L)

    tril = np.tril(np.ones((L, L), dtype=bool))
    if np.array_equal(mask, tril):
        causal = True
    elif mask.all():
        causal = False
    else:
        raise NotImplementedError("only causal or all-true masks supported")

    in_maps = []
    for c in range(NCORES):
        b, g = divmod(c, 2)
        heads = np.arange(HC * g, HC * (g + 1))
        band = _make_band(rel_emb, heads, causal)       # [128, BAND_W, HC]
        band = np.ascontiguousarray(band.transpose(0, 2, 1))  # [128, HC, W]
        bfut = np.broadcast_to(
            (rel_emb[0][heads] - rel_emb[NUM_BUCKETS - 1][heads])
            .astype(np.float32), (128, HC)).copy()
        in_maps.append(dict(
            xT=np.ascontiguousarray(inputs[b].T),
            wq=np.ascontiguousarray(wq[:, heads, :]).reshape(E, HC * D),
            wk=np.ascontiguousarray(wk[:, heads, :]).reshape(E, HC * D),
            wv=np.ascontiguousarray(wv[:, heads, :]).reshape(E, HC * D),
            wo=np.ascontiguousarray(wo[heads]).reshape(HC * D, E),
            wi=np.ascontiguousarray(wi[:, FC * g:FC * (g + 1)]),
            wmo=np.ascontiguousarray(wmo[FC * g:FC * (g + 1), :]),
            band=band,
            bfut=bfut,
        ))
    return in_maps, causal, inputs


def run(trace=False, **kw):
    in_maps, causal, inputs = _prep_in_maps(**kw)
    nc = _get_nc(causal)
    res = run_bass_kernel_spmd(nc, in_maps, list(range(NCORES)), trace=trace)
    out = np.empty((B, L, E), dtype=np.float32)
    for b in range(B):
        out[b] = (inputs[b]
                  + res.results[2 * b]["attn_out"]
                  + res.results[2 * b]["mlp_out"]
                  + res.results[2 * b + 1]["attn_out"]
                  + res.results[2 * b + 1]["mlp_out"])
    return out, res


def kernel(**inputs):
    out, _ = run(**inputs)
    return out
